# revision 1
# baseline (speedup 1.0000x reference)
"""BitLinear v3: transpose-free. Host ships signsT pre-transposed AND pre-packed
into the exact SBUF image (fp8 +/-1, per-chunk contiguous per partition) so DMA
descriptors are large. Device does orientation-B matmuls (lhsT = raw sign tile,
rhs = xT) into a b-major PSUM tile via strided writes; dequant+reduce on drain:
  yT[o,b] = sum_g scale[o,g] * (sT_g.T @ xT_g)[o,b]
Per block: 32 matmuls -> psum [r,32b,32g], one TT-mult by scale (middle
broadcast over b), one contiguous innermost reduce over g -> y_sb[r, b, :].
Output is yT [1376, 32] per core; host transposes and concatenates.
"""

import numpy as np

BATCH = 32
IN_F = 4096
OUT_F = 11008
GROUP = 128
N_GROUPS = IN_F // GROUP  # 32
N_CORES = 8
O_SHARD = OUT_F // N_CORES  # 1376
N_BLOCKS = (O_SHARD + 127) // 128  # 11 (10 full + 96 remainder)
CHUNK_O = 256  # o-columns per DMA chunk (2 blocks)
N_CHUNKS = (O_SHARD + CHUNK_O - 1) // CHUNK_O  # 6 (last = 96 wide)
IMG_F = N_GROUPS * O_SHARD  # 44032 free bytes per partition (fp8)

SIGN_DT = "bf16"  # prescaled weights

_nc_cache = []


def _chunk_widths():
    return [min(CHUNK_O, O_SHARD - c * CHUNK_O) for c in range(N_CHUNKS)]


def build_nc():
    import concourse.bacc as bacc
    import concourse.mybir as mybir
    import concourse.tile as tile
    from concourse.masks import make_identity

    f32 = mybir.dt.float32
    bf16 = mybir.dt.bfloat16
    sdt = mybir.dt.float8e4 if SIGN_DT == "fp8" else bf16

    nc = bacc.Bacc(None, target_bir_lowering=False)
    x_d = nc.dram_tensor("x", [BATCH, IN_F], f32, kind="ExternalInput")
    sT_d = nc.dram_tensor("signsT", [128, IMG_F], sdt, kind="ExternalInput")
    y_d = nc.dram_tensor("y", [O_SHARD, BATCH], f32, kind="ExternalOutput")

    with tile.TileContext(nc) as tc:
        with tc.tile_pool(name="const", bufs=1) as const, tc.tile_pool(
            name="psum", bufs=1, space="PSUM"
        ) as psum:
            ident = const.tile([128, 128], bf16, tag="ident")
            make_identity(nc, ident)

            x_sb = const.tile([BATCH, IN_F], f32, tag="x_sb")
            x_bf = const.tile([BATCH, IN_F], bf16, tag="x_bf")
            xT = const.tile([128, N_GROUPS, BATCH], bf16, tag="xT")
            y_sb = const.tile([128, N_BLOCKS, BATCH], f32, tag="y_sb")

            nc.sync.dma_start(x_sb[:], x_d[:])
            nc.vector.tensor_copy(x_bf[:], x_sb[:])
            for half in range(2):
                xp = psum.tile([128, 16, BATCH], bf16, tag="xp", bufs=2)
                for c in range(16):
                    g = half * 16 + c
                    nc.tensor.transpose(
                        xp[:, c, :],
                        x_bf[:, g * GROUP : (g + 1) * GROUP],
                        ident[:BATCH, :BATCH],
                    )
                nc.vector.tensor_copy(xT[:, half * 16 : (half + 1) * 16, :], xp[:])

            s_chunks = []
            off = 0
            for c, w in enumerate(_chunk_widths()):
                sc = const.tile([128, N_GROUPS, w], sdt, tag=f"sT{c}")
                n = 2
                gs = N_GROUPS // n
                span = gs * w
                for q in range(n):
                    nc.sync.dma_start(
                        sc[:, q * gs : (q + 1) * gs, :],
                        sT_d[:, off + q * span : off + (q + 1) * span].rearrange(
                            "p (g o) -> p g o", g=gs
                        ),
                    )
                off += N_GROUPS * w
                s_chunks.append(sc)

            # per block: 32 accumulating matmuls into one [r, b] psum tile,
            # then a single tiny copy out -- no dequant drain at all
            for b in range(N_BLOCKS):
                r = min(128, O_SHARD - b * 128)
                sc = s_chunks[b // 2]
                oc = (b % 2) * 128
                ps = psum.tile([128, BATCH], f32, tag="ps", bufs=2)
                for g in range(N_GROUPS):
                    nc.tensor.matmul(
                        ps[:r, :],
                        sc[:, g, oc : oc + r],
                        xT[:, g, :],
                        start=(g == 0),
                        stop=(g == N_GROUPS - 1),
                    )
                nc.vector.tensor_copy(y_sb[:r, b, :], ps[:r, :])

            nc.sync.dma_start(
                y_d[0 : 10 * 128].rearrange("(blk p) b -> p blk b", p=128),
                y_sb[:, 0:10, :],
            )
            nc.sync.dma_start(y_d[10 * 128 : O_SHARD], y_sb[:96, 10, :])
    nc.finalize()
    return nc


def _pack_signs(signs_shard, scales_shard):
    """[O_SHARD, IN_F] +/-1 and [O_SHARD, N_GROUPS] -> prescaled bf16 SBUF
    image [128, IMG_F], per-chunk contiguous per partition."""
    import ml_dtypes

    np_dt = ml_dtypes.bfloat16
    w_full = signs_shard.astype(np.float32) * np.repeat(
        scales_shard.astype(np.float32), GROUP, axis=1
    )
    sT = w_full.T  # [IN_F, O_SHARD]
    img = np.empty((128, IMG_F), dtype=np_dt)
    off = 0
    o0 = 0
    for w in _chunk_widths():
        sub = sT[:, o0 : o0 + w].reshape(N_GROUPS, 128, w)
        img[:, off : off + N_GROUPS * w] = (
            sub.transpose(1, 0, 2).reshape(128, N_GROUPS * w).astype(np_dt)
        )
        off += N_GROUPS * w
        o0 += w
    return img


def _shard_inputs(x, scales, signs):
    scales_r = scales.reshape(OUT_F, N_GROUPS)
    x32 = np.ascontiguousarray(x, dtype=np.float32)
    in_maps = []
    for c in range(N_CORES):
        lo, hi = c * O_SHARD, (c + 1) * O_SHARD
        in_maps.append(
            {
                "x": x32,
                "signsT": _pack_signs(signs[lo:hi], scales_r[lo:hi]),
            }
        )
    return in_maps


def _run(x, scales, signs, trace=False, tmpdir=None):
    from concourse import bass_utils

    if not _nc_cache:
        _nc_cache.append(build_nc())
    nc = _nc_cache[0]
    in_maps = _shard_inputs(x, scales, signs)
    res = bass_utils.run_bass_kernel_spmd(
        nc, in_maps, list(range(N_CORES)), trace=trace, tmpdir=tmpdir
    )
    out = np.concatenate(
        [np.asarray(res.results[i]["y"]).T for i in range(N_CORES)], axis=1
    )
    return np.ascontiguousarray(out).astype(np.float32), res


def kernel(x, scales, signs):
    out, _ = _run(x, scales, signs)
    return out



# revision 2
# speedup vs baseline: 1.4097x; 1.4097x over previous
"""BitLinear v4: x-stationary / weight-streaming with fp8(e3m4) weights.

Orientation: out[b, o] = sum_k x[b,k] w[o,k] computed as 32 accumulating
matmuls (one per k-group g) with lhsT = xT_g [128k, 32b] (stationary, bf16)
and rhs = W_g [128k, O] (moving, fp8e3m4 prescaled sign*scale*64).  The big
operand STREAMS through the PE at 1 col/cycle instead of being loaded as
stationary, and fp8 halves HBM traffic vs bf16 (the memory-regime wall).

Col-tiling: O_SHARD=1376 is split into 4 chunks of 344; tile_position=(0,32j)
packs 4 concurrent M=32 matmuls into the 128-wide PE array, each streaming its
own o-chunk, accumulating into PSUM partitions 32j..32j+32.  Drain is a single
[128,344] copy + DMA; host reassembles [4,32,344] -> [32,1376] per core.

Numerics: e4m3 scale quantization fails the 2e-2 gate (3 mantissa bits,
measured 0.029); e3m4 (4 mantissa bits, range [0.25,15.5]) with scales
renormalized by 64 (s*64 in [0.32,3.2], all normal) measures 0.014.  The /64
compensation is folded into the host-side bf16 x image (exact exponent shift).
"""

import numpy as np

BATCH = 32
IN_F = 4096
OUT_F = 11008
GROUP = 128
N_GROUPS = IN_F // GROUP  # 32
N_CORES = 8
O_SHARD = OUT_F // N_CORES  # 1376
N_OTILE = 4
O_TILE = O_SHARD // N_OTILE  # 344
W_IMG_F = N_GROUPS * O_SHARD  # 44032 fp8 bytes per partition
N_WSLICES = 16
G_PER_SLICE = N_GROUPS // N_WSLICES  # 2
SCALE_NORM = 64.0  # w' = sign*scale*64 (e3m4 normal range), x' = x/64

_nc_cache = []


def build_nc():
    import concourse.bacc as bacc
    import concourse.mybir as mybir
    import concourse.tile as tile

    f32 = mybir.dt.float32
    bf16 = mybir.dt.bfloat16
    fp8 = mybir.dt.float8e3

    nc = bacc.Bacc(None, target_bir_lowering=False)
    x_d = nc.dram_tensor("xT", [128, N_GROUPS * BATCH], bf16, kind="ExternalInput")
    w_d = nc.dram_tensor("wT", [128, W_IMG_F], fp8, kind="ExternalInput")
    y_d = nc.dram_tensor("y", [128, O_TILE], f32, kind="ExternalOutput")

    with tile.TileContext(nc) as tc:
        with tc.tile_pool(name="const", bufs=1) as const, tc.tile_pool(
            name="psum", bufs=1, space="PSUM"
        ) as psum:
            x_sb = const.tile([128, N_GROUPS, BATCH], bf16, tag="x_sb")
            w_sb = const.tile([128, N_GROUPS, O_SHARD], fp8, tag="w_sb")
            y_sb = const.tile([128, O_TILE], f32, tag="y_sb")

            nc.sync.dma_start(
                x_sb[:], x_d[:].rearrange("p (g b) -> p g b", g=N_GROUPS)
            )
            span = G_PER_SLICE * O_SHARD
            for s in range(N_WSLICES):
                nc.sync.dma_start(
                    w_sb[:, s * G_PER_SLICE : (s + 1) * G_PER_SLICE, :],
                    w_d[:, s * span : (s + 1) * span].rearrange(
                        "p (g o) -> p g o", g=G_PER_SLICE
                    ),
                )

            ps = psum.tile([128, O_TILE], f32, tag="ps")
            for g in range(N_GROUPS):
                for j in range(N_OTILE):
                    nc.tensor.matmul(
                        ps[32 * j : 32 * (j + 1), :],
                        x_sb[:, g, :],
                        w_sb[:, g, j * O_TILE : (j + 1) * O_TILE],
                        start=(g == 0),
                        stop=(g == N_GROUPS - 1),
                        tile_position=(0, 32 * j),
                    )
            nc.vector.tensor_copy(y_sb[:], ps[:])
            nc.sync.dma_start(y_d[:], y_sb[:])
    nc.finalize()
    return nc


def _pack_weights(signs_shard, scales_shard):
    """[O_SHARD, IN_F] +/-1 int and [O_SHARD, N_GROUPS] f32 -> e3m4 image
    [128, W_IMG_F]: img[p, 1376*g + o] = sign[o, 128g+p]*scale[o,g]*64."""
    import ml_dtypes

    w = signs_shard.astype(np.float32) * np.repeat(
        scales_shard.astype(np.float32) * SCALE_NORM, GROUP, axis=1
    )  # [O_SHARD, IN_F]
    # -> [g, p, o] -> [p, g, o] -> [128, W_IMG_F]
    img = (
        w.T.reshape(N_GROUPS, GROUP, O_SHARD)
        .transpose(1, 0, 2)
        .reshape(128, W_IMG_F)
        .astype(ml_dtypes.float8_e3m4)
    )
    return img


def _pack_x(x):
    """[32, 4096] f32 -> bf16 image [128, N_GROUPS*BATCH]:
    img[p, 32*g + b] = x[b, 128g+p] / 64."""
    import ml_dtypes

    return np.ascontiguousarray(
        (x.astype(np.float32).T / SCALE_NORM)
        .reshape(N_GROUPS, GROUP, BATCH)
        .transpose(1, 0, 2)
        .reshape(128, N_GROUPS * BATCH)
        .astype(ml_dtypes.bfloat16)
    )


def _shard_inputs(x, scales, signs):
    scales_r = np.asarray(scales).reshape(OUT_F, N_GROUPS)
    signs = np.asarray(signs)
    x_img = _pack_x(np.asarray(x))
    in_maps = []
    for c in range(N_CORES):
        lo, hi = c * O_SHARD, (c + 1) * O_SHARD
        in_maps.append(
            {
                "xT": x_img,
                "wT": _pack_weights(signs[lo:hi], scales_r[lo:hi]),
            }
        )
    return in_maps


def _run(x, scales, signs, trace=False, tmpdir=None):
    from concourse import bass_utils

    if not _nc_cache:
        _nc_cache.append(build_nc())
    nc = _nc_cache[0]
    in_maps = _shard_inputs(x, scales, signs)
    res = bass_utils.run_bass_kernel_spmd(
        nc, in_maps, list(range(N_CORES)), trace=trace, tmpdir=tmpdir
    )
    # per core: y [128, 344] where partition 32j+b holds out[b, o_tile j]
    parts = []
    for i in range(N_CORES):
        yc = np.asarray(res.results[i]["y"]).reshape(N_OTILE, 32, O_TILE)
        parts.append(yc.transpose(1, 0, 2).reshape(BATCH, O_SHARD))
    out = np.concatenate(parts, axis=1)
    return np.ascontiguousarray(out).astype(np.float32), res


def kernel(x, scales, signs):
    out, _ = _run(x, scales, signs)
    return out
